# revision 6
# baseline (speedup 1.0000x reference)
"""Trainium2 Bass kernel for nn_Aggregate (segment_reduce).

Reference computation:
    cat_idx = idx_inputs[:, argmax(softmax(cat_mask))]          # [N]
    agg     = segment_sum(inputs[:, 16:], cat_idx, 100000)       # [S, 128]
    out     = agg[cat_idx][:, top32(softmax(numeric_mask))] * conf

Strategy (v6 -- one custom paged-sum DVE op does everything):
  * Only the 32 top-k numeric columns survive to the output, and segment_sum
    is linear per column -> select those 32 columns FIRST (4x less data)
    and fold the conf scaling into them.
  * Sort rows by segment on the host.  Each segment is one contiguous run.
  * Runs are padded to an even number of rows (split into 2 interleaved
    "phase" planes), bucketed by half-length l = ceil(len/2), and dealt
    uniformly to 8 cores x 4 partition-groups (dummy runs pad each bucket
    to a multiple of 32), so each bucket region is a regular [q pages x l
    cols] grid, identical on every core/group.
  * A hand-assembled custom DVE op (PAGED_PAIR_SUM) consumes both phase
    streams as a [128, q, l] paged access pattern at 1 column/cycle:
    state += in0[c] + in1[c] in fp32, the state RESETS at every page
    (= run) boundary via a 1-cycle reseed bubble on SUB_DIM_DONE, and
    only the page totals are written (out_last_subdim_enable) -- the op
    emits the compacted per-segment sums [128, q] fp16 directly.  No
    masks, no prefix-sum buffer, no extraction pass, no collectives.
  * Host does only routing (sort, bucket, deal, gather); every add that
    touches row data happens on the device.

Everything data-dependent (bucket geometry, window cuts) is baked into the
compiled graph; build_bass() therefore runs after prepare().
"""

import copy
import sys
import types

import ml_dtypes
import numpy as np

if "/opt/trn_rl_repo" not in sys.path:
    sys.path.insert(0, "/opt/trn_rl_repo")

import concourse.bacc as bacc
import concourse.dve_ops as dve_ops
import concourse.mybir as mybir
import concourse.tile as tile
from concourse.dve_spec import C0, Spec, Src0, Src1
from concourse.dve_spec import AluOp as DveAluOp
from concourse.dve_spec import lower
from concourse.dve_spec import scan as dve_scan
from concourse.dve_uop import DveOpSpec, Trigger

# ----------------------------------------------------------------------------
# problem constants (hardcoded per spec)
N_ROWS = 1_000_000
NUM_CAT = 16
NUM_NUMERICS = 128
N_ARY = 32
NUM_SEGMENTS = 100_000

NCORES = 8
GROUPS = 4                    # partition-groups per core (32 feats each)
NSTREAM = NCORES * GROUPS     # 32 independent streams
PH = 2                        # phase planes (the op adds both per cycle)
# window fractions (/32 of C2): small first window -> DVE starts early;
# small last window -> short DVE drain + tiny final output DMA after the
# input stream ends.  The whole input stays resident in SBUF, so every
# window's DMA is issued up front and the queues never drain.
WFRAC = (1, 3, 3, 3, 3, 3, 3, 3, 3, 3, 3, 1)
NWIN = len(WFRAC)
# merge a bucket into the next length while the padding cost (cols per
# stream) stays below the ~0.45us per-instance DVE overhead equivalent
MERGE_COST_COLS = 250

BF16 = ml_dtypes.bfloat16
F16 = np.float16

_dt = mybir.dt

_CACHE: dict = {}


def _paged_ref(in0, in1, s0, s1, imm2):
    x = in0.astype(np.float32) + in1.astype(np.float32)
    return x.sum(axis=-1) + np.asarray(s0, dtype=np.float32)


def _register_paged_pair_sum():
    """Custom DVE op: out[p, j] = s0 + sum_c (in0[p, j, c] + in1[p, j, c]).

    Hand-assembled 3-uop program (seed / steady / reseed) derived from the
    stock-lowered scan(ADD, Src0+Src1, init=C0):
      - steady runs the one-cycle-recurrence scan at 1 column/cycle and
        writes ONLY each page's last value (out_last_subdim_enable);
      - at every SUB_DIM_DONE (page boundary of the [P, q, l] access
        pattern) a 1-cycle non-consuming reseed bubble re-primes the
        recurrence register with C0, exactly like the initial seed.
    The per-NEFF uop table ships via the ant.dve_table HLO
    frontend-attribute path; the compile cache is pre-seeded so the DSL
    lowering (which cannot express per-page resets) is bypassed."""
    name = "PAGED_PAIR_SUM_AGG"
    for op in dve_ops.OPS:
        if op.name == name:
            return op
    spec = Spec(
        body=dve_scan(DveAluOp.ADD, Src0 + Src1, init=C0),
        reference=_paged_ref,
    )
    row = max(dve_ops._SUB_OPCODE_FOR_NAME.values()) + 1
    op = dve_ops.DveOp(name, spec, subdim=True, uops_sha={})
    dve_ops.OPS.append(op)
    dve_ops.CUSTOM_DVE_SPECS[name] = spec
    dve_ops._SUB_OPCODE_FOR_NAME[name] = row

    seed, steady = lower(
        Spec(body=dve_scan(DveAluOp.ADD, Src0 + Src1, init=C0)), ver="v3")
    seed2 = copy.deepcopy(seed)
    steady2 = copy.deepcopy(steady)
    steady2.trigger = (Trigger.SRC_TENSOR_DONE, Trigger.SUB_DIM_DONE,
                      Trigger.NONE)
    steady2.next_uop = (0, 2, 0)          # done -> IDLE, page end -> reseed
    steady2.out_last_subdim_enable = 1    # emit only page totals
    reseed = copy.deepcopy(seed)
    reseed.next_uop = (1, 0, 0)           # back to steady
    uops = [seed2, steady2, reseed]
    for ver in ("v3", "v4"):
        dve_ops._COMPILE_CACHE[(name, ver)] = DveOpSpec(
            name=name, opcode=row, uops=uops, rd1_en=True)
    return op


_PAGED = _register_paged_pair_sum()


def _ensure_axon_hooks():
    """bass_utils imports antenv.axon_hooks for trace=True; provide a shim
    so the import never fails (hook stays None unless a profiler sets it)."""
    if "antenv.axon_hooks" in sys.modules:
        return sys.modules["antenv.axon_hooks"]
    mod = types.ModuleType("antenv.axon_hooks")
    hook = [None]
    mod.set_axon_ntff_profile_hook = lambda h: hook.__setitem__(0, h)
    mod.get_axon_ntff_profile_hook = lambda: hook[0]
    sys.modules["antenv.axon_hooks"] = mod
    return mod


def _softmax64(v):
    v = np.asarray(v, dtype=np.float64)
    e = np.exp(v - v.max())
    return e / e.sum()


def prepare(inputs, idx_inputs, cat_mask, numeric_mask):
    """Host-side prep: top-k, column select + conf scale, sort, bucket by
    half run length, deal runs to 32 streams, build phase planes, cut
    page-aligned windows.

    Returns (in_maps, meta); stashes the device-graph geometry in
    _CACHE["geo"] for build_bass().
    """
    cat_mask = np.asarray(cat_mask)
    numeric_mask = np.asarray(numeric_mask)
    cm = _softmax64(cat_mask)
    ti = int(np.argmax(cm))                     # top_k(1) -> first max
    top_cat_val = cm[ti]
    nm = _softmax64(numeric_mask)
    order = np.argsort(-nm, kind="stable")[:N_ARY]   # descending, ties->low idx
    conf = ((nm[order] + top_cat_val) / 2.0).astype(np.float32)

    seg = np.ascontiguousarray(np.asarray(idx_inputs)[:, ti]).astype(np.int32)
    perm = np.argsort(seg, kind="stable")
    seg_s = seg[perm]

    inputs = np.asarray(inputs)
    sel = inputs[:, NUM_CAT + order].astype(np.float32) * conf[None, :]
    xs = sel[perm].astype(BF16)                  # [N, 32] sorted rows, bf16

    # ---- run bookkeeping ----------------------------------------------
    isstart = np.empty(N_ROWS, dtype=bool)
    isstart[0] = True
    isstart[1:] = seg_s[1:] != seg_s[:-1]
    rank_s = np.cumsum(isstart) - 1              # [N] run index of each row
    start_pos = np.flatnonzero(isstart)          # [R]
    nruns = len(start_pos)
    lens = np.empty(nruns, dtype=np.int64)
    lens[:-1] = np.diff(start_pos)
    lens[-1] = N_ROWS - start_pos[-1]
    seg_of_run = seg_s[start_pos]                # [R]
    lp = (lens + PH - 1) // PH                   # page length per run

    # ---- bucket by page length, deal to 32 streams --------------------
    # stream s <-> (core = s // GROUPS, group = s % GROUPS)
    blens = np.unique(lp)
    s_of_run = np.empty(nruns, dtype=np.int64)
    k_of_run = np.empty(nruns, dtype=np.int64)   # page index within bucket
    bkt_of_run = np.empty(nruns, dtype=np.int64)
    buckets = []                                 # (l, q, B, O) per bucket
    base = 0
    out_base = 0
    for bi, l in enumerate(blens):
        ridx = np.flatnonzero(lp == l)
        m = len(ridx)
        q = -(-m // NSTREAM)                     # pages per stream
        s_of_run[ridx] = np.arange(m) % NSTREAM
        k_of_run[ridx] = np.arange(m) // NSTREAM
        bkt_of_run[ridx] = bi
        buckets.append((int(l), int(q), int(base), int(out_base)))
        base += q * l
        out_base += q
    C2 = base
    Q = out_base

    bucket_B = np.array([b[2] for b in buckets], dtype=np.int64)
    bucket_O = np.array([b[3] for b in buckets], dtype=np.int64)
    bucket_L = np.array([b[0] for b in buckets], dtype=np.int64)
    off_of_run = bucket_B[bkt_of_run] + k_of_run * bucket_L[bkt_of_run]
    outcol_of_run = bucket_O[bkt_of_run] + k_of_run

    # ---- page-aligned window cuts and per-window op instances ---------
    frac = np.cumsum(WFRAC)
    denom = int(frac[-1])
    bounds = [0]
    for k in range(1, NWIN):
        t = C2 * int(frac[k - 1]) // denom
        cut = C2
        for (l, q, B, O) in buckets:
            if B <= t < B + q * l:
                j = (t - B + l // 2) // l
                cut = B + j * l
                break
            if t < B:
                cut = B
                break
        bounds.append(max(bounds[-1], min(cut, C2)))
    bounds.append(C2)
    # instances ordered (window, bucket); output columns are assigned in
    # this order so each window's outputs form one contiguous [q0, q1)
    # slice that can be DMA'd out as soon as the window's DVE ops finish.
    instances = []          # (w, local_off, n_pages, l, out_off_new)
    perm = np.full(Q, -1, dtype=np.int64)   # old outcol -> new outcol
    wq = [0]                # per-window output column ranges
    newcol = 0
    for w in range(NWIN):
        for (l, q, B, O) in buckets:
            lo = max(B, bounds[w])
            hi = min(B + q * l, bounds[w + 1])
            if lo >= hi:
                continue
            j0 = (lo - B) // l
            j1 = (hi - B) // l
            n = j1 - j0
            instances.append((w, lo - bounds[w], n, l, newcol))
            perm[O + j0:O + j1] = np.arange(newcol, newcol + n)
            newcol += n
        wq.append(newcol)
    assert newcol == Q and (perm >= 0).all()
    WL = max(bounds[w + 1] - bounds[w] for w in range(NWIN))

    # ---- scatter sorted rows into per-stream phase-resolved planes ----
    big = np.zeros((NSTREAM, C2 * PH, N_ARY), dtype=BF16)
    within = np.arange(N_ROWS, dtype=np.int64) - start_pos[rank_s]
    srow = s_of_run[rank_s]
    posrow = PH * off_of_run[rank_s] + within
    big.reshape(-1, N_ARY)[srow * (C2 * PH) + posrow] = xs

    # [NSTREAM, C2, PH, 32] -> [NSTREAM, 32feat, PH, C2]
    planes = big.reshape(NSTREAM, C2, PH, N_ARY)
    planes = np.ascontiguousarray(planes.transpose(0, 3, 2, 1))

    # window-contiguous layout: per stream/feature the window w block is
    # [phase0 cols ws:we | phase1 cols ws:we], so one contiguous DMA per
    # window moves both phase streams.
    flat = np.empty((NSTREAM, N_ARY, 2 * C2), dtype=BF16)
    for w in range(NWIN):
        ws, we = bounds[w], bounds[w + 1]
        flat[:, :, 2 * ws:2 * we] = planes[:, :, :, ws:we].reshape(
            NSTREAM, N_ARY, 2 * (we - ws))

    planes8 = flat.reshape(NCORES, 128, 2 * C2)
    in_maps = []
    for i in range(NCORES):
        in_maps.append({"xin": planes8[i].view(np.uint8)})

    _CACHE["geo"] = {"C2": C2, "Q": Q, "bounds": bounds,
                     "instances": instances, "WL": WL, "buckets": buckets,
                     "wq": wq}
    meta = {
        "seg": seg,
        "seg_of_run": seg_of_run,
        "core_of_run": s_of_run // GROUPS,
        "group_of_run": s_of_run % GROUPS,
        "outcol_of_run": perm[outcol_of_run],
        "Q": Q,
    }
    return in_maps, meta


def build_bass():
    """Build + compile the (SPMD, per-core identical) Bass graph.

    Geometry (window cuts, paged-op instances) comes from prepare()'s
    stash, so prepare() must run first.
    """
    if "nc" in _CACHE:
        return _CACHE["nc"]
    geo = _CACHE["geo"]
    C2, Q, bounds, instances, WL, wq = (geo["C2"], geo["Q"], geo["bounds"],
                                        geo["instances"], geo["WL"],
                                        geo["wq"])

    nc = bacc.Bacc("TRN2", target_bir_lowering=False, debug=False,
                   num_devices=NCORES)
    xin = nc.dram_tensor("xin", [128, C2 * 4], _dt.uint8,
                         kind="ExternalInput").ap()
    xout = nc.dram_tensor("out", [128, Q * 2], _dt.uint8,
                          kind="ExternalOutput").ap()

    by_win = {w: [] for w in range(NWIN)}
    for inst in instances:
        by_win[inst[0]].append(inst)

    with tile.TileContext(nc) as tc:
        with tc.tile_pool(name="pp", bufs=1) as pool:
            ot = pool.tile([128, Q], _dt.float16, tag="o")
            # the whole input lives in SBUF (2*C2 bf16 = ~66KB/partition),
            # so all window DMAs are issued up front and the SDMA queues
            # stream back-to-back with no DVE-gated refill stalls.
            xt = pool.tile([128, 2 * C2], _dt.bfloat16, tag="x")
            for w in range(NWIN):
                ws, we = bounds[w], bounds[w + 1]
                wlen = we - ws
                base = 2 * ws
                # one contiguous DMA per window carries both phase streams;
                # alternate HWDGE queues so windows drain round-robin.
                ieng = nc.sync if w % 2 == 0 else nc.scalar
                ieng.dma_start(
                    out=xt[:, base:2 * we],
                    in_=xin[:, ws * 4:we * 4].bitcast(_dt.bfloat16))
                for (_, off, n, l, oo) in by_win[w]:
                    nc.vector._custom_dve(
                        _PAGED,
                        out=ot[:, oo:oo + n],
                        in0=xt[:, base + off:base + off + n * l].rearrange(
                            "p (q l) -> p q l", l=l),
                        in1=xt[:, base + wlen + off:
                               base + wlen + off + n * l].rearrange(
                            "p (q l) -> p q l", l=l),
                        s0=0.0)
                # stream this window's (contiguous) outputs out while later
                # windows are still loading; gpsimd keeps the HWDGE queues
                # free for input issue, the last window uses an idle HWDGE
                # engine for its lower fixed latency.
                q0, q1 = wq[w], wq[w + 1]
                if q1 > q0:
                    if w < NWIN - 1:
                        oeng = nc.gpsimd
                    else:
                        oeng = nc.scalar if w % 2 == 0 else nc.sync
                    oeng.dma_start(out=xout[:, q0 * 2:q1 * 2],
                                   in_=ot[:, q0:q1].bitcast(_dt.uint8))
    nc.compile()
    _CACHE["nc"] = nc
    return nc


def postprocess(results, meta):
    """Pull per-run sums from the compacted device outputs, expand to rows."""
    table = np.zeros((NUM_SEGMENTS, N_ARY), dtype=np.float32)
    core = meta["core_of_run"]
    group = meta["group_of_run"]
    outcol = meta["outcol_of_run"]
    for i in range(NCORES):
        O = results[i]["out"].view(F16).astype(np.float32)       # [128, Q]
        O = O.reshape(GROUPS, 32, meta["Q"])
        m = core == i
        table[meta["seg_of_run"][m]] = O[group[m], :, outcol[m]]
    return table[meta["seg"]]


def run(in_maps, trace=False, trace_kwargs=None):
    _ensure_axon_hooks()
    from concourse.bass_utils import run_bass_kernel_spmd
    nc = build_bass()
    return run_bass_kernel_spmd(nc, in_maps, core_ids=list(range(NCORES)),
                                trace=trace, **(trace_kwargs or {}))


def kernel(inputs, idx_inputs, cat_mask, numeric_mask):
    in_maps, meta = prepare(inputs, idx_inputs, cat_mask, numeric_mask)
    res = run(in_maps, trace=False)
    return postprocess(res.results, meta)



# revision 8
# speedup vs baseline: 1.0735x; 1.0735x over previous
"""Trainium2 Bass kernel for nn_Aggregate (segment_reduce).

Reference computation:
    cat_idx = idx_inputs[:, argmax(softmax(cat_mask))]          # [N]
    agg     = segment_sum(inputs[:, 16:], cat_idx, 100000)       # [S, 128]
    out     = agg[cat_idx][:, top32(softmax(numeric_mask))] * conf

Strategy (v6 -- one custom paged-sum DVE op does everything):
  * Only the 32 top-k numeric columns survive to the output, and segment_sum
    is linear per column -> select those 32 columns FIRST (4x less data)
    and fold the conf scaling into them.
  * Sort rows by segment on the host.  Each segment is one contiguous run.
  * Runs are padded to an even number of rows (split into 2 interleaved
    "phase" planes), bucketed by half-length l = ceil(len/2), and dealt
    uniformly to 8 cores x 4 partition-groups (dummy runs pad each bucket
    to a multiple of 32), so each bucket region is a regular [q pages x l
    cols] grid, identical on every core/group.
  * A hand-assembled custom DVE op (PAGED_PAIR_SUM) consumes both phase
    streams as a [128, q, l] paged access pattern at 1 column/cycle:
    state += in0[c] + in1[c] in fp32, the state RESETS at every page
    (= run) boundary via a 1-cycle reseed bubble on SUB_DIM_DONE, and
    only the page totals are written (out_last_subdim_enable) -- the op
    emits the compacted per-segment sums [128, q] fp16 directly.  No
    masks, no prefix-sum buffer, no extraction pass, no collectives.
  * Host does only routing (sort, bucket, deal, gather); every add that
    touches row data happens on the device.

Everything data-dependent (bucket geometry, window cuts) is baked into the
compiled graph; build_bass() therefore runs after prepare().
"""

import copy
import sys
import types

import ml_dtypes
import numpy as np

if "/opt/trn_rl_repo" not in sys.path:
    sys.path.insert(0, "/opt/trn_rl_repo")

import concourse.bacc as bacc
import concourse.dve_ops as dve_ops
import concourse.mybir as mybir
import concourse.tile as tile
from concourse.dve_spec import C0, Spec, Src0, Src1
from concourse.dve_spec import AluOp as DveAluOp
from concourse.dve_spec import lower
from concourse.dve_spec import scan as dve_scan
from concourse.dve_uop import DveOpSpec, Trigger

# ----------------------------------------------------------------------------
# problem constants (hardcoded per spec)
N_ROWS = 1_000_000
NUM_CAT = 16
NUM_NUMERICS = 128
N_ARY = 32
NUM_SEGMENTS = 100_000

NCORES = 8
GROUPS = 4                    # partition-groups per core (32 feats each)
NSTREAM = NCORES * GROUPS     # 32 independent streams
PH = 2                        # phase planes (the op adds both per cycle)
# window fractions (/32 of C2): small first window -> DVE starts early;
# small last window -> short DVE drain + tiny final output DMA after the
# input stream ends.  The whole input stays resident in SBUF; each window
# is split into two half-DMAs (one per HWDGE queue) so windows complete
# strictly in issue order at the full aggregate DMA rate.
WFRAC = (2, 5, 5, 6, 6, 5, 2, 1)
NWIN = len(WFRAC)
# windows after which the accumulated output columns are streamed out
# (SWDGE/gpsimd for the mid-stream flushes; the tiny final flush goes on
# an idle HWDGE engine for its lower fixed latency)
OUT_FLUSH = (3, 6, 7)
# merge a bucket into the next length while the padding cost (cols per
# stream) stays below the ~0.45us per-instance DVE overhead equivalent
MERGE_COST_COLS = 250

BF16 = ml_dtypes.bfloat16
F16 = np.float16

_dt = mybir.dt

_CACHE: dict = {}


def _paged_ref(in0, in1, s0, s1, imm2):
    x = in0.astype(np.float32) + in1.astype(np.float32)
    return x.sum(axis=-1) + np.asarray(s0, dtype=np.float32)


def _register_paged_pair_sum():
    """Custom DVE op: out[p, j] = s0 + sum_c (in0[p, j, c] + in1[p, j, c]).

    Hand-assembled 3-uop program (seed / steady / reseed) derived from the
    stock-lowered scan(ADD, Src0+Src1, init=C0):
      - steady runs the one-cycle-recurrence scan at 1 column/cycle and
        writes ONLY each page's last value (out_last_subdim_enable);
      - at every SUB_DIM_DONE (page boundary of the [P, q, l] access
        pattern) a 1-cycle non-consuming reseed bubble re-primes the
        recurrence register with C0, exactly like the initial seed.
    The per-NEFF uop table ships via the ant.dve_table HLO
    frontend-attribute path; the compile cache is pre-seeded so the DSL
    lowering (which cannot express per-page resets) is bypassed."""
    name = "PAGED_PAIR_SUM_AGG"
    for op in dve_ops.OPS:
        if op.name == name:
            return op
    spec = Spec(
        body=dve_scan(DveAluOp.ADD, Src0 + Src1, init=C0),
        reference=_paged_ref,
    )
    row = max(dve_ops._SUB_OPCODE_FOR_NAME.values()) + 1
    op = dve_ops.DveOp(name, spec, subdim=True, uops_sha={})
    dve_ops.OPS.append(op)
    dve_ops.CUSTOM_DVE_SPECS[name] = spec
    dve_ops._SUB_OPCODE_FOR_NAME[name] = row

    seed, steady = lower(
        Spec(body=dve_scan(DveAluOp.ADD, Src0 + Src1, init=C0)), ver="v3")
    seed2 = copy.deepcopy(seed)
    steady2 = copy.deepcopy(steady)
    steady2.trigger = (Trigger.SRC_TENSOR_DONE, Trigger.SUB_DIM_DONE,
                      Trigger.NONE)
    steady2.next_uop = (0, 2, 0)          # done -> IDLE, page end -> reseed
    steady2.out_last_subdim_enable = 1    # emit only page totals
    reseed = copy.deepcopy(seed)
    reseed.next_uop = (1, 0, 0)           # back to steady
    uops = [seed2, steady2, reseed]
    for ver in ("v3", "v4"):
        dve_ops._COMPILE_CACHE[(name, ver)] = DveOpSpec(
            name=name, opcode=row, uops=uops, rd1_en=True)
    return op


_PAGED = _register_paged_pair_sum()


def _ensure_axon_hooks():
    """bass_utils imports antenv.axon_hooks for trace=True; provide a shim
    so the import never fails (hook stays None unless a profiler sets it)."""
    if "antenv.axon_hooks" in sys.modules:
        return sys.modules["antenv.axon_hooks"]
    mod = types.ModuleType("antenv.axon_hooks")
    hook = [None]
    mod.set_axon_ntff_profile_hook = lambda h: hook.__setitem__(0, h)
    mod.get_axon_ntff_profile_hook = lambda: hook[0]
    sys.modules["antenv.axon_hooks"] = mod
    return mod


def _softmax64(v):
    v = np.asarray(v, dtype=np.float64)
    e = np.exp(v - v.max())
    return e / e.sum()


def prepare(inputs, idx_inputs, cat_mask, numeric_mask):
    """Host-side prep: top-k, column select + conf scale, sort, bucket by
    half run length, deal runs to 32 streams, build phase planes, cut
    page-aligned windows.

    Returns (in_maps, meta); stashes the device-graph geometry in
    _CACHE["geo"] for build_bass().
    """
    cat_mask = np.asarray(cat_mask)
    numeric_mask = np.asarray(numeric_mask)
    cm = _softmax64(cat_mask)
    ti = int(np.argmax(cm))                     # top_k(1) -> first max
    top_cat_val = cm[ti]
    nm = _softmax64(numeric_mask)
    order = np.argsort(-nm, kind="stable")[:N_ARY]   # descending, ties->low idx
    conf = ((nm[order] + top_cat_val) / 2.0).astype(np.float32)

    seg = np.ascontiguousarray(np.asarray(idx_inputs)[:, ti]).astype(np.int32)
    perm = np.argsort(seg, kind="stable")
    seg_s = seg[perm]

    inputs = np.asarray(inputs)
    sel = inputs[:, NUM_CAT + order].astype(np.float32) * conf[None, :]
    xs = sel[perm].astype(BF16)                  # [N, 32] sorted rows, bf16

    # ---- run bookkeeping ----------------------------------------------
    isstart = np.empty(N_ROWS, dtype=bool)
    isstart[0] = True
    isstart[1:] = seg_s[1:] != seg_s[:-1]
    rank_s = np.cumsum(isstart) - 1              # [N] run index of each row
    start_pos = np.flatnonzero(isstart)          # [R]
    nruns = len(start_pos)
    lens = np.empty(nruns, dtype=np.int64)
    lens[:-1] = np.diff(start_pos)
    lens[-1] = N_ROWS - start_pos[-1]
    seg_of_run = seg_s[start_pos]                # [R]
    lp = (lens + PH - 1) // PH                   # page length per run

    # ---- bucket by page length, deal to 32 streams --------------------
    # stream s <-> (core = s // GROUPS, group = s % GROUPS)
    blens = np.unique(lp)
    s_of_run = np.empty(nruns, dtype=np.int64)
    k_of_run = np.empty(nruns, dtype=np.int64)   # page index within bucket
    bkt_of_run = np.empty(nruns, dtype=np.int64)
    buckets = []                                 # (l, q, B, O) per bucket
    base = 0
    out_base = 0
    for bi, l in enumerate(blens):
        ridx = np.flatnonzero(lp == l)
        m = len(ridx)
        q = -(-m // NSTREAM)                     # pages per stream
        s_of_run[ridx] = np.arange(m) % NSTREAM
        k_of_run[ridx] = np.arange(m) // NSTREAM
        bkt_of_run[ridx] = bi
        buckets.append((int(l), int(q), int(base), int(out_base)))
        base += q * l
        out_base += q
    C2 = base
    Q = out_base

    bucket_B = np.array([b[2] for b in buckets], dtype=np.int64)
    bucket_O = np.array([b[3] for b in buckets], dtype=np.int64)
    bucket_L = np.array([b[0] for b in buckets], dtype=np.int64)
    off_of_run = bucket_B[bkt_of_run] + k_of_run * bucket_L[bkt_of_run]
    outcol_of_run = bucket_O[bkt_of_run] + k_of_run

    # ---- page-aligned window cuts and per-window op instances ---------
    frac = np.cumsum(WFRAC)
    denom = int(frac[-1])
    bounds = [0]
    for k in range(1, NWIN):
        t = C2 * int(frac[k - 1]) // denom
        cut = C2
        for (l, q, B, O) in buckets:
            if B <= t < B + q * l:
                j = (t - B + l // 2) // l
                cut = B + j * l
                break
            if t < B:
                cut = B
                break
        bounds.append(max(bounds[-1], min(cut, C2)))
    bounds.append(C2)
    # instances ordered (window, bucket); output columns are assigned in
    # this order so each window's outputs form one contiguous [q0, q1)
    # slice that can be DMA'd out as soon as the window's DVE ops finish.
    instances = []          # (w, local_off, n_pages, l, out_off_new)
    perm = np.full(Q, -1, dtype=np.int64)   # old outcol -> new outcol
    wq = [0]                # per-window output column ranges
    newcol = 0
    for w in range(NWIN):
        for (l, q, B, O) in buckets:
            lo = max(B, bounds[w])
            hi = min(B + q * l, bounds[w + 1])
            if lo >= hi:
                continue
            j0 = (lo - B) // l
            j1 = (hi - B) // l
            n = j1 - j0
            instances.append((w, lo - bounds[w], n, l, newcol))
            perm[O + j0:O + j1] = np.arange(newcol, newcol + n)
            newcol += n
        wq.append(newcol)
    assert newcol == Q and (perm >= 0).all()
    WL = max(bounds[w + 1] - bounds[w] for w in range(NWIN))

    # ---- scatter sorted rows into per-stream phase-resolved planes ----
    big = np.zeros((NSTREAM, C2 * PH, N_ARY), dtype=BF16)
    within = np.arange(N_ROWS, dtype=np.int64) - start_pos[rank_s]
    srow = s_of_run[rank_s]
    posrow = PH * off_of_run[rank_s] + within
    big.reshape(-1, N_ARY)[srow * (C2 * PH) + posrow] = xs

    # [NSTREAM, C2, PH, 32] -> [NSTREAM, 32feat, PH, C2]
    planes = big.reshape(NSTREAM, C2, PH, N_ARY)
    planes = np.ascontiguousarray(planes.transpose(0, 3, 2, 1))

    # window-contiguous layout: per stream/feature the window w block is
    # [phase0 cols ws:we | phase1 cols ws:we], so one contiguous DMA per
    # window moves both phase streams.
    flat = np.empty((NSTREAM, N_ARY, 2 * C2), dtype=BF16)
    for w in range(NWIN):
        ws, we = bounds[w], bounds[w + 1]
        flat[:, :, 2 * ws:2 * we] = planes[:, :, :, ws:we].reshape(
            NSTREAM, N_ARY, 2 * (we - ws))

    planes8 = flat.reshape(NCORES, 128, 2 * C2)
    in_maps = []
    for i in range(NCORES):
        in_maps.append({"xin": planes8[i].view(np.uint8)})

    _CACHE["geo"] = {"C2": C2, "Q": Q, "bounds": bounds,
                     "instances": instances, "WL": WL, "buckets": buckets,
                     "wq": wq}
    meta = {
        "seg": seg,
        "seg_of_run": seg_of_run,
        "core_of_run": s_of_run // GROUPS,
        "group_of_run": s_of_run % GROUPS,
        "outcol_of_run": perm[outcol_of_run],
        "Q": Q,
    }
    return in_maps, meta


def build_bass():
    """Build + compile the (SPMD, per-core identical) Bass graph.

    Geometry (window cuts, paged-op instances) comes from prepare()'s
    stash, so prepare() must run first.
    """
    if "nc" in _CACHE:
        return _CACHE["nc"]
    geo = _CACHE["geo"]
    C2, Q, bounds, instances, WL, wq = (geo["C2"], geo["Q"], geo["bounds"],
                                        geo["instances"], geo["WL"],
                                        geo["wq"])

    nc = bacc.Bacc("TRN2", target_bir_lowering=False, debug=False,
                   num_devices=NCORES)
    xin = nc.dram_tensor("xin", [128, C2 * 4], _dt.uint8,
                         kind="ExternalInput").ap()
    xout = nc.dram_tensor("out", [128, Q * 2], _dt.uint8,
                          kind="ExternalOutput").ap()

    by_win = {w: [] for w in range(NWIN)}
    for inst in instances:
        by_win[inst[0]].append(inst)

    with tile.TileContext(nc) as tc:
        with tc.tile_pool(name="pp", bufs=1) as pool:
            ot = pool.tile([128, Q], _dt.float16, tag="o")
            # the whole input lives in SBUF (2*C2 bf16 = ~66KB/partition),
            # so all window DMAs are issued up front and the SDMA queues
            # stream back-to-back with no DVE-gated refill stalls.
            xt = pool.tile([128, 2 * C2], _dt.bfloat16, tag="x")
            flushed = 0
            for w in range(NWIN):
                ws, we = bounds[w], bounds[w + 1]
                wlen = we - ws
                base = 2 * ws
                # phase-split halves, one per HWDGE queue: both halves drain
                # concurrently, so window w completes at cum_bytes(w)/rate.
                nc.sync.dma_start(
                    out=xt[:, base:base + wlen],
                    in_=xin[:, ws * 4:ws * 4 + wlen * 2].bitcast(
                        _dt.bfloat16))
                nc.scalar.dma_start(
                    out=xt[:, base + wlen:2 * we],
                    in_=xin[:, ws * 4 + wlen * 2:we * 4].bitcast(
                        _dt.bfloat16))
                for (_, off, n, l, oo) in by_win[w]:
                    nc.vector._custom_dve(
                        _PAGED,
                        out=ot[:, oo:oo + n],
                        in0=xt[:, base + off:base + off + n * l].rearrange(
                            "p (q l) -> p q l", l=l),
                        in1=xt[:, base + wlen + off:
                               base + wlen + off + n * l].rearrange(
                            "p (q l) -> p q l", l=l),
                        s0=0.0)
                # stream accumulated outputs out while later windows load
                if w in OUT_FLUSH:
                    q0, q1 = flushed, wq[w + 1]
                    flushed = q1
                    if q1 > q0:
                        oeng = nc.gpsimd if w < NWIN - 1 else nc.sync
                        oeng.dma_start(out=xout[:, q0 * 2:q1 * 2],
                                       in_=ot[:, q0:q1].bitcast(_dt.uint8))
    nc.compile()
    _CACHE["nc"] = nc
    return nc


def postprocess(results, meta):
    """Pull per-run sums from the compacted device outputs, expand to rows."""
    table = np.zeros((NUM_SEGMENTS, N_ARY), dtype=np.float32)
    core = meta["core_of_run"]
    group = meta["group_of_run"]
    outcol = meta["outcol_of_run"]
    for i in range(NCORES):
        O = results[i]["out"].view(F16).astype(np.float32)       # [128, Q]
        O = O.reshape(GROUPS, 32, meta["Q"])
        m = core == i
        table[meta["seg_of_run"][m]] = O[group[m], :, outcol[m]]
    return table[meta["seg"]]


def run(in_maps, trace=False, trace_kwargs=None):
    _ensure_axon_hooks()
    from concourse.bass_utils import run_bass_kernel_spmd
    nc = build_bass()
    return run_bass_kernel_spmd(nc, in_maps, core_ids=list(range(NCORES)),
                                trace=trace, **(trace_kwargs or {}))


def kernel(inputs, idx_inputs, cat_mask, numeric_mask):
    in_maps, meta = prepare(inputs, idx_inputs, cat_mask, numeric_mask)
    res = run(in_maps, trace=False)
    return postprocess(res.results, meta)

